# revision 22
# baseline (speedup 1.0000x reference)
"""MoE (top-2 of 8 experts) Trainium2 Bass kernel, data-parallel over tokens on 8 cores.

Contract: kernel(**inputs) takes the FULL fp32 inputs (hidden_states [4,4096,1024],
w_gate [8,1024], w_fc [8,2048,1024], b_fc [8,2048], w_proj [8,1024,2048],
b_proj [8,1024]) and returns the FULL [4,4096,1024] fp32 output.

Strategy:
  - 8 cores, each owns 2048 tokens and replicates all 8 experts' weights.
  - The token->core assignment and the token->expert dispatch layout are decided
    on the host as part of sharding (balanced round-robin over expert-pair
    types; per-expert token blocks are host-gathered into dense fp16 dispatch
    buffers, host provides the scatter-index tables for the combine).
  - All NN *values* are computed on device: per-slot router logits are
    recomputed from the dispatched activations (wg matmul + PE transpose), the
    top-2 softmax gate = sigmoid(l_sel - l_oth) on ACT, expert FC (fp16 matmul
    + exact-gelu) and PROJ (fp16 matmul), per-token gate scale on DVE, and a
    dma_scatter_add combine into the pre-zeroed output (capacity-pad slots land
    on a dump row and are discarded).
  - Per-expert capacities are exact host counts rounded to 64 (the dispatch is
    host-authoritative, so no safety margin is needed).
"""

import math
import numpy as np
from contextlib import ExitStack

import concourse.bass as bass
import concourse.bacc as bacc
import concourse.mybir as mybir
import concourse.tile as tile
from concourse import bass_utils

F32 = mybir.dt.float32
F16 = mybir.dt.float16
I16 = mybir.dt.int16
U32 = mybir.dt.uint32

N_CORES = 8
B, S, H, I = 4, 4096, 1024, 2048
E, TOPK = 8, 2
T = B * S              # 16384 total tokens
TC = T // N_CORES      # 2048 tokens per core
HC = H // 128          # 8 h-chunks
IC = I // 128          # 16 i-chunks


def _n_chunks(total, step=512):
    """Split `total` into near-equal chunks of at most `step` columns (each a
    multiple of 32) — balanced chunks avoid the small-N matmul issue floor."""
    n = (total + step - 1) // step
    per = ((-(-total // n) + 31) // 32) * 32
    out = []
    o = 0
    while o < total:
        out.append((o, min(per, total - o)))
        o += per
    return out


def build_program(caps):
    """Build the SPMD per-core program. caps: per-expert compute capacity
    (multiple of 64) — FC/PROJ/scatter process this many token slots."""
    nc = bacc.Bacc("TRN2", target_bir_lowering=False, debug=False,
                   num_devices=N_CORES)

    ntmax = max((c + 127) // 128 for c in caps)
    wgT = nc.dram_tensor("wgT", [H, E], F16, kind="ExternalInput")
    ident = nc.dram_tensor("ident", [E, E], F32, kind="ExternalInput")
    wfcT = nc.dram_tensor("wfcT", [E, 4, H, I // 4], F16, kind="ExternalInput")
    wpjT = nc.dram_tensor("wpjT", [E, I, H], F16, kind="ExternalInput")
    bfcT = nc.dram_tensor("bfcT", [E, 128, IC], F32, kind="ExternalInput")
    bpjB = nc.dram_tensor("bpjB", [E, 128, H], F32, kind="ExternalInput")
    xeb = [nc.dram_tensor(f"xeb{e}", [128, HC, caps[e]], F16,
                          kind="ExternalInput") for e in range(E)]
    # scatter targets per slot (16-wrapped int16; pads -> dump row TC)
    sidx = [nc.dram_tensor(f"sidx{e}", [128, caps[e] // 16], I16,
                           kind="ExternalInput") for e in range(E)]
    # per-slot logit-difference masks: dm[p, t, k] = +1 for the slot's own
    # expert, -1 for the token's other selected expert (0 rows for pads)
    dmm = [nc.dram_tensor(f"dm{e}", [128, ntmax, 8], F32,
                          kind="ExternalInput") for e in range(E)]
    # +128 dump rows: capacity-pad entries scatter there and are discarded
    out = nc.dram_tensor("out", [TC + 128, H], F16, kind="ExternalOutput")

    # experts processed largest first; the smallest runs last so the final
    # scatter tail is short
    order = sorted(range(E), key=lambda e: (-caps[e], e))

    with tile.TileContext(nc) as tc, ExitStack() as ctx:
        const_pool = ctx.enter_context(tc.tile_pool(name="const", bufs=1))
        wfc_pool = ctx.enter_context(tc.tile_pool(name="wfc", bufs=2))
        wpj_pool = ctx.enter_context(tc.tile_pool(name="wpj", bufs=1))
        xe_pool = ctx.enter_context(tc.tile_pool(name="xe", bufs=2))
        bias_pool = ctx.enter_context(tc.tile_pool(name="bias", bufs=2))
        sidx_pool = ctx.enter_context(tc.tile_pool(name="sidx", bufs=E))
        dm_pool = ctx.enter_context(tc.tile_pool(name="dm", bufs=E))
        lg_pool = ctx.enter_context(tc.tile_pool(name="lg", bufs=2))
        gc_pool = ctx.enter_context(tc.tile_pool(name="gc", bufs=2))
        hm_pool = ctx.enter_context(tc.tile_pool(name="hm", bufs=1))
        y_pool = ctx.enter_context(tc.tile_pool(name="y", bufs=3))
        psf_pool = ctx.enter_context(tc.tile_pool(name="psf", bufs=3, space="PSUM"))
        psp_pool = ctx.enter_context(tc.tile_pool(name="psp", bufs=2, space="PSUM"))
        psl_pool = ctx.enter_context(tc.tile_pool(name="psl", bufs=2, space="PSUM"))
        pst_pool = ctx.enter_context(tc.tile_pool(name="pst", bufs=1, space="PSUM"))

        wfc_t, wpj_t, bias_t, xe_t, sidx_t, dm_t = {}, {}, {}, {}, {}, {}

        def load_xeb(e, ring=None):
            cap = caps[e]
            xe = xe_pool.tile([128, HC, cap], F16, tag="xe", name=f"xe{e}")
            (ring or nc.sync).dma_start(xe[:], xeb[e].ap())
            xe_t[e] = xe

        def load_bias(e):
            bfc = bias_pool.tile([128, IC], F32, tag="bfc", name=f"bfc{e}")
            nc.sync.dma_start(bfc[:], bfcT.ap()[e])
            bpj = bias_pool.tile([128, H], F32, tag="bpj", name=f"bpj{e}")
            nc.sync.dma_start(bpj[:], bpjB.ap()[e])
            bias_t[e] = (bfc, bpj)

        def load_wfc(e, rings=None):
            grp = []
            for g in range(4):
                wf = wfc_pool.tile([128, HC, I // 4], F16, tag=f"wfc{g}",
                                   name=f"wfc{e}g{g}")
                ring = rings[g] if rings else nc.sync
                ring.dma_start(
                    wf[:], wfcT.ap()[e][g].rearrange("(c p) i -> p c i", p=128))
                grp.append(wf)
            wfc_t[e] = grp

        def load_wpj(e, ring=None):
            wpj = wpj_pool.tile([128, IC, H], F16, tag="wpj", name=f"wpj{e}")
            (ring or nc.sync).dma_start(
                wpj[:], wpjT.ap()[e].rearrange("(c p) h -> p c h", p=128))
            wpj_t[e] = wpj

        # ---------------- Prologue -------------------------------------------
        # priority DMA, balanced across the two HWDGE queues so FC(order[0])
        # can start as soon as possible: ACT queue carries the first tokens +
        # next expert's FC weights; SP queue carries the first FC weights,
        # then constants, then the first PROJ weights
        wg_sb = const_pool.tile([128, HC, E], F16)
        nc.sync.dma_start(wg_sb[:],
                          wgT.ap().rearrange("(c p) e -> p c e", p=128))
        id_sb = const_pool.tile([E, E], F32)
        nc.sync.dma_start(id_sb[:], ident.ap())
        load_xeb(order[0], ring=nc.scalar)
        load_wfc(order[0], rings=[nc.sync, nc.scalar, nc.sync, nc.scalar])
        load_xeb(order[1], ring=nc.scalar)
        load_wfc(order[1], rings=[nc.scalar, nc.sync, nc.scalar, nc.sync])
        for e in range(E):
            st = sidx_pool.tile([128, caps[e] // 16], I16, tag="sx",
                                name=f"sx{e}")
            nc.sync.dma_start(st[:], sidx[e].ap())
            sidx_t[e] = st
            dt = dm_pool.tile([128, ntmax, 8], F32, tag="dm", name=f"dm{e}")
            nc.sync.dma_start(dt[:], dmm[e].ap())
            dm_t[e] = dt
        load_bias(order[0])
        load_bias(order[1])
        load_wpj(order[0], ring=nc.sync)

        # PE warmup (~6us of dummy matmuls: opens the HAM clock gate) + prime
        # the ACT tables (Sigmoid, Gelu) while the first inputs DMA in
        wu = const_pool.tile([128, 128], F16)
        nc.vector.memset(wu[:], 0.0)
        wps = psp_pool.tile([128, 512], F32, tag="psp")
        for _ in range(200):
            nc.tensor.matmul(wps[:, :128], wu[:], wu[:], start=True, stop=True)
        wug = const_pool.tile([128, 2], F32)
        nc.scalar.activation(wug[:, 0:1], wu[:, 0:1],
                             mybir.ActivationFunctionType.Sigmoid)
        nc.scalar.activation(wug[:, 1:2], wu[:, 1:2],
                             mybir.ActivationFunctionType.Gelu)
        # preload the scatter q7 library (+pay its IRAM load) off the critical
        # path: scatter a zero tile onto the dump row
        zt = const_pool.tile([128, 1, H], F16)
        nc.vector.memset(zt[:], 0.0)
        zi = const_pool.tile([128, 8], I16)
        nc.vector.memset(zi[:], TC)
        nc.gpsimd.dma_scatter_add(out.ap(), zt[:], zi[:], 128, 128, H)

        # ---------------- Per-expert: gates + FC + PROJ + scatter ------------
        for i, e in enumerate(order):
            cap = caps[e]
            nt = (cap + 127) // 128
            # prefetch: later experts' tokens and weights while this computes
            if i + 2 < E:
                load_xeb(order[i + 2])
                load_wfc(order[i + 2])
            if i + 1 < E and order[i + 1] not in bias_t:
                load_bias(order[i + 1])
            if i + 1 < E and order[i + 1] not in wpj_t:
                load_wpj(order[i + 1])
            xe = xe_t.pop(e)
            wfc = wfc_t.pop(e)
            wpj = wpj_t.pop(e)
            bfc, bpj = bias_t.pop(e)

            # router logits for this expert's slots: l_all[k, slot] =
            # sum_h wgT[h, k] * xeb[h, slot], then per-128-slot PE transpose
            lsb = lg_pool.tile([8, cap], F32, tag="lsb")
            for (n0, nlen) in _n_chunks(cap):
                pl = psl_pool.tile([8, 512], F32, tag="psl")
                for hc in range(HC):
                    nc.tensor.matmul(pl[:, :nlen], wg_sb[:, hc, :],
                                     xe[:, hc, n0:n0 + nlen],
                                     start=(hc == 0), stop=(hc == HC - 1))
                nc.vector.tensor_copy(lsb[:, n0:n0 + nlen], pl[:, :nlen])
            lT = lg_pool.tile([128, nt, 8], F32, tag="lT")
            if cap % 128:
                nc.vector.memset(lT[cap % 128:, nt - 1, :], 0.0)
            for tt in range(nt):
                tk = min(128, cap - tt * 128)
                pt = pst_pool.tile([128, 8], F32, tag="pst")
                nc.tensor.transpose(pt[:tk, :],
                                    lsb[:, tt * 128:tt * 128 + tk], id_sb[:])
                nc.vector.tensor_copy(lT[:tk, tt, :], pt[:tk, :])
            # gate[slot] = sigmoid(l_sel - l_oth)  (= top-2 softmax weight)
            gd = gc_pool.tile([128, nt, 8], F32, tag="gd")
            nc.vector.tensor_mul(gd[:], lT[:], dm_t[e][:, 0:nt, :])
            gci = gc_pool.tile([128, nt, 1], F32, tag="gci")
            nc.vector.tensor_reduce(gci[:], gd[:], axis=mybir.AxisListType.X,
                                    op=mybir.AluOpType.add)
            gcol = gc_pool.tile([128, nt, 1], F32, tag="gc")
            nc.scalar.activation(gcol[:], gci[:],
                                 mybir.ActivationFunctionType.Sigmoid)

            # FC: hmid[i, tok] = gelu(sum_h wfcT[h,i] * x_t[h,tok] + b_fc[i])
            hm = hm_pool.tile([128, IC, cap], F16, tag="hm")
            for ic in range(IC):
                for (n0, nlen) in _n_chunks(cap):
                    ps = psf_pool.tile([128, 512], F32, tag="psf")
                    for hc in range(HC):
                        nc.tensor.matmul(
                            ps[:, :nlen],
                            wfc[ic // 4][:, hc, (ic % 4) * 128:(ic % 4 + 1) * 128],
                            xe[:, hc, n0:n0 + nlen],
                            start=(hc == 0), stop=(hc == HC - 1))
                    nc.scalar.activation(
                        hm[:, ic, n0:n0 + nlen], ps[:, :nlen],
                        mybir.ActivationFunctionType.Gelu,
                        bias=bfc[:, ic:ic + 1])

            # PROJ: y[tok, h] = sum_i hmid[i, tok] * wprojT[i, h]; then (y+b)*g
            y = y_pool.tile([128, nt, H], F16, tag="y")
            if cap % 128:
                # partial last tile: the scatter reads all 128 partitions
                # (only num_idxs rows are sent); zero the unwritten tail
                nc.vector.memset(y[cap % 128:, nt - 1, :], 0.0)
            for tt in range(nt):
                tk = min(128, cap - tt * 128)
                for (h0, hlen) in _n_chunks(H):
                    ps = psp_pool.tile([128, 512], F32, tag="psp")
                    for ic in range(IC):
                        nc.tensor.matmul(
                            ps[:tk, :hlen],
                            hm[:, ic, tt * 128:tt * 128 + tk],
                            wpj[:, ic, h0:h0 + hlen],
                            start=(ic == 0), stop=(ic == IC - 1))
                    ysl = y[:tk, tt, h0:h0 + hlen]
                    nc.vector.tensor_add(ysl, ps[:tk, :hlen],
                                         bpj[:tk, h0:h0 + hlen])
                    nc.vector.tensor_scalar_mul(ysl, ysl, gcol[:tk, tt, 0:1])
                # scatter this token tile as soon as it's scaled
                nc.gpsimd.dma_scatter_add(out.ap(), y[:, tt:tt + 1, :],
                                          sidx_t[e][:, tt * 8:tt * 8 + tk // 16],
                                          tk, tk, H)

    nc.compile()
    return nc


def _host_routing(x2d, w_gate):
    """Host-side routing: top-2 picks (ordered top1-first)."""
    logits = x2d.astype(np.float32) @ w_gate.astype(np.float32).T  # [T, E]
    order = np.argsort(-logits, axis=-1)
    return order[:, :2]                                            # [T, 2]


def _balanced_perm(top2):
    """Token permutation: round-robin each expert-pair type across cores so
    per-(core,expert) counts land within a few tokens of global/8."""
    pair_id = top2.min(axis=1) * E + top2.max(axis=1)
    grouped = np.argsort(pair_id, kind="stable")
    core_of = np.empty(T, dtype=np.int64)
    core_of[grouped] = np.arange(T) % N_CORES
    perm = np.argsort(core_of, kind="stable")
    return perm


_PROGRAM_CACHE = {}


def _get_program(caps):
    key = tuple(caps)
    if key not in _PROGRAM_CACHE:
        _PROGRAM_CACHE[key] = build_program(key)
    return _PROGRAM_CACHE[key]


def make_in_maps(hidden_states, w_gate, w_fc, b_fc, w_proj, b_proj):
    """Host-side shard + dispatch layout. Returns (in_maps, caps, perm)."""
    x2d = np.asarray(hidden_states, dtype=np.float32).reshape(T, H)
    w_gate = np.asarray(w_gate, dtype=np.float32)
    w_fc = np.asarray(w_fc, dtype=np.float32)
    b_fc = np.asarray(b_fc, dtype=np.float32)
    w_proj = np.asarray(w_proj, dtype=np.float32)
    b_proj = np.asarray(b_proj, dtype=np.float32)

    top2 = _host_routing(x2d, w_gate)
    perm = _balanced_perm(top2)
    counts = np.zeros((N_CORES, E), dtype=np.int64)
    for c in range(N_CORES):
        np.add.at(counts[c], top2[perm[c * TC:(c + 1) * TC]].ravel(), 1)
    caps = tuple(int(math.ceil(n / 64.0) * 64) for n in counts.max(axis=0))
    ntmax = max((c + 127) // 128 for c in caps)

    wgT = np.ascontiguousarray(w_gate.T).astype(np.float16)    # [H, E]
    ident = np.eye(E, dtype=np.float32)
    wfcT = np.ascontiguousarray(
        w_fc.transpose(0, 2, 1).reshape(E, H, 4, I // 4)
        .transpose(0, 2, 1, 3)).astype(np.float16)
    wpjT = np.ascontiguousarray(w_proj.transpose(0, 2, 1)).astype(np.float16)
    bfcT = np.ascontiguousarray(b_fc.reshape(E, IC, 128).transpose(0, 2, 1))
    bpjB = np.ascontiguousarray(
        np.broadcast_to(b_proj[:, None, :], (E, 128, H)))

    in_maps = []
    for c in range(N_CORES):
        tok = perm[c * TC:(c + 1) * TC]
        xc = x2d[tok]                                          # [TC, H]
        t2 = top2[tok]                                         # [TC, 2]
        m = {"wgT": wgT, "ident": ident, "wfcT": wfcT, "wpjT": wpjT,
             "bfcT": bfcT, "bpjB": bpjB}
        for e in range(E):
            cap = caps[e]
            sel = np.where((t2 == e).any(axis=1))[0]           # local token ids
            n_e = len(sel)
            assert n_e <= cap
            blk = np.zeros((cap, H), dtype=np.float16)
            blk[:n_e] = xc[sel]
            m[f"xeb{e}"] = np.ascontiguousarray(
                blk.T.reshape(HC, 128, cap).transpose(1, 0, 2))
            flat = np.full(cap, TC, dtype=np.int16)
            flat[:n_e] = sel
            sx = flat.reshape(cap // 16, 16).T            # slot s -> [s%16, s//16]
            m[f"sidx{e}"] = np.ascontiguousarray(np.tile(sx, (8, 1)))
            dm = np.zeros((128, ntmax, 8), dtype=np.float32)
            oth = np.where(t2[sel, 0] == e, t2[sel, 1], t2[sel, 0])
            s = np.arange(n_e)
            dm[s % 128, s // 128, e] += 1.0
            dm[s % 128, s // 128, oth] -= 1.0
            m[f"dm{e}"] = dm
        in_maps.append(m)
    return in_maps, caps, perm


def _ensure_ntff_hook():
    """This image's antenv lacks axon_hooks; bridge it so trace=True works."""
    import sys
    import types
    try:
        import antenv.axon_hooks  # noqa: F401
        return
    except ImportError:
        pass
    hook = None
    try:
        from trn_agent_boot.trn_boot import _ntff_profile_via_ctypes
        hook = _ntff_profile_via_ctypes("/opt/axon/libaxon_pjrt.so")
    except Exception:
        pass
    mod = types.ModuleType("antenv.axon_hooks")
    state = {"hook": hook}
    mod.get_axon_ntff_profile_hook = lambda: state["hook"]
    mod.set_axon_ntff_profile_hook = lambda h: state.update(hook=h)
    sys.modules["antenv.axon_hooks"] = mod
    try:
        import antenv
        antenv.axon_hooks = mod
    except ImportError:
        pass


def kernel(hidden_states, w_gate, w_fc, b_fc, w_proj, b_proj,
           _trace=False, _tmpdir=None):
    if _trace:
        _ensure_ntff_hook()
    in_maps, caps, perm = make_in_maps(
        hidden_states, w_gate, w_fc, b_fc, w_proj, b_proj)
    nc = _get_program(caps)
    res = bass_utils.run_bass_kernel_spmd(
        nc, in_maps, core_ids=list(range(N_CORES)),
        trace=_trace, tmpdir=_tmpdir)
    shuf = np.concatenate([res.results[c]["out"][:TC] for c in range(N_CORES)],
                          axis=0)
    outp = np.empty_like(shuf)
    outp[perm] = shuf
    kernel.last_results = res
    return outp.reshape(B, S, H).astype(np.float32)
